# revision 23
# baseline (speedup 1.0000x reference)
"""Pure-gather LoRA embedding kernel, raw-bass Block version.

Folded table (host: W + 2*B@A -> fp16), then on-device per core:
16 indirect DMAs gather 128 rows each DIRECTLY from the table in HBM to
the output in HBM (no SBUF staging, no stores). Raw Block (no
TileContext) keeps the Pool instruction queue free of per-DMA semaphore
bookkeeping, so gathers issue back-to-back at the Q7's intrinsic pitch.

Set D2D=False to fall back to SBUF staging + HWDGE stores.
"""

import numpy as np

try:
    import concourse.bass as bass
except ImportError:
    import sys

    sys.path.insert(0, "/opt/trn_rl_repo")
    import concourse.bass as bass

import concourse.mybir as mybir
from concourse import bacc
from concourse.bass_utils import run_bass_kernel_spmd

VOCAB = 50257
DIM = 1024
SCALING = 32.0 / 16.0
N_CORES = 8
TOK_PER_CORE = 2048
P = 128
N_TILES = TOK_PER_CORE // P

D2D = False

_cached_nc = None


def _indirect_d2d(g, out_ap, in_ap, off_ap):
    """indirect_dma_start with a DRAM destination (bass asserts SBUF;
    this is the same lowering without that assert)."""
    out_l = g.lower_ap_dma(out_ap, for_indirect_dma=True)
    in_l = g.lower_ap_dma(in_ap, for_indirect_dma=True)
    assert len(in_l) == 1 and len(out_l) == 1
    off_l = g.lower_ap_dma(off_ap)
    assert len(off_l) == 1
    in_l.append(off_l[0])

    coef = in_ap.shape[1]  # elements per table row
    dynamic_ap_info = mybir.DynamicAccessPatternInfo(
        c=0,
        actual_ap=out_l[0].ap,
        indirect_dim_max_index=in_ap.shape[0],
        offset_expr=[
            mybir.DynamicAccessPatternOffsetExpr(
                coef=coef,
                aff_expr=mybir.DynamicAccessPatternOffsetExprAffExpr(
                    kind="IndirectArgId", arg_id=1
                ),
            )
        ],
    )
    in_l[0].dynamic_ap_info = dynamic_ap_info
    return g.add_instruction(
        mybir.InstDMACopy(
            name=g.bass.get_next_instruction_name(),
            queue="qPoolDynamic",
            mode="Copy",
            ins=in_l,
            outs=out_l,
            oob_is_err=True,
            cce_op=mybir.AluOpType.bypass,
        )
    )


def _build_nc():
    global _cached_nc
    if _cached_nc is not None:
        return _cached_nc

    f16 = mybir.dt.float16
    nc = bacc.Bacc(None, target_bir_lowering=False, dynamic_dma_scratch_size=65536)
    ids_d = nc.declare_dram_parameter("ids", [P, N_TILES], mybir.dt.int32, isOutput=False)
    t_d = nc.declare_dram_parameter("table", [VOCAB, DIM], f16, isOutput=False)
    out_d = nc.declare_dram_parameter("out", [TOK_PER_CORE, DIM], f16, isOutput=True)

    from contextlib import ExitStack

    with (
        nc.Block() as block,
        nc.sbuf_tensor("ids_sb", [P, N_TILES], mybir.dt.int32) as ids_sb,
        nc.sbuf_tensor("stage", [P, N_TILES * DIM], f16) as stage,
        nc.semaphore("io") as io_sem,
        nc.semaphore("sto") as sto_sem,
        ExitStack() as stack,
    ):
        gsems = [
            stack.enter_context(nc.semaphore(f"g{j}"))  # noqa: ANT232
            for j in range(N_TILES)
        ]

        @block.sync
        def _(sync: bass.BassEngine):
            sync.dma_start(ids_sb[:], ids_d[:]).then_inc(io_sem, 16)
            if not D2D:
                # even tiles; odd tiles store via the ACT HWDGE ring so the
                # tail's wait->store chains run on two engines in parallel
                for j in range(0, N_TILES, 2):
                    sync.wait_ge(gsems[j], 16)
                    sync.dma_start(
                        out_d[j * P : (j + 1) * P, :],
                        stage[:, j * DIM : (j + 1) * DIM],
                    ).then_inc(sto_sem, 16)
                sync.wait_ge(sto_sem, 16 * N_TILES)

        @block.scalar
        def _(sc: bass.BassEngine):
            if not D2D:
                for j in range(1, N_TILES, 2):
                    sc.wait_ge(gsems[j], 16)
                    sc.dma_start(
                        out_d[j * P : (j + 1) * P, :],
                        stage[:, j * DIM : (j + 1) * DIM],
                    ).then_inc(sto_sem, 16)

        @block.gpsimd
        def _(g: bass.BassGpSimd):
            g.wait_ge(io_sem, 16)
            for j in range(N_TILES):
                off = ids_sb.ap()[:, j : j + 1]
                if D2D:
                    _indirect_d2d(
                        g, out_d[j * P : (j + 1) * P, :], t_d[:], off
                    ).then_inc(gsems[j], 16)
                else:
                    g.indirect_dma_start(
                        out=stage.ap()[:, j * DIM : (j + 1) * DIM],
                        out_offset=None,
                        in_=t_d[:],
                        in_offset=bass.IndirectOffsetOnAxis(ap=off, axis=0),
                    ).then_inc(gsems[j], 16)
            if D2D:
                for j in range(N_TILES):
                    g.wait_ge(gsems[j], 16)

    nc.compile()
    _cached_nc = nc
    return nc


def prepare(inputs):
    ids = np.ascontiguousarray(
        np.asarray(inputs["input_ids"]).astype(np.int32)
    ).reshape(-1)
    weight = np.asarray(inputs["weight"], dtype=np.float32)
    lora_a = np.ascontiguousarray(np.asarray(inputs["lora_A"], dtype=np.float32))
    lora_b = np.asarray(inputs["lora_B"], dtype=np.float32)

    table = (weight + SCALING * (lora_b @ lora_a)).astype(np.float16)

    nc = _build_nc()
    in_maps = []
    for c in range(N_CORES):
        chunk = ids[c * TOK_PER_CORE : (c + 1) * TOK_PER_CORE]
        ids_dev = np.ascontiguousarray(chunk.reshape(N_TILES, P).T)
        in_maps.append({"ids": ids_dev, "table": table})
    return in_maps, nc


def postprocess_core(out_core, core_idx):
    return out_core


def run(inputs, **spmd_kwargs):
    in_maps, nc = prepare(inputs)
    res = run_bass_kernel_spmd(nc, in_maps, list(range(N_CORES)), **spmd_kwargs)
    out = np.stack([res.results[c]["out"] for c in range(N_CORES)], axis=0)
    return out.astype(np.float32), res


def kernel(**inputs):
    out, _ = run(inputs)
    return out
